# revision 1
# baseline (speedup 1.0000x reference)
"""Bayesian linear layer (reparameterized sample + predictive uncertainty)
as an 8-core SPMD Trainium2 Bass kernel.

Reference computation (all fp32):
    W     = weight_mu + exp(weight_log_sigma) * eps_w          # [OUT, IN]
    b     = bias_mu + exp(bias_log_sigma) * eps_b              # [OUT]
    out   = x @ W.T + b                                        # [B, OUT]
    unc   = sqrt((x*x) @ (exp(weight_log_sigma)**2).T + exp(bias_log_sigma)**2)

Sharding: 2 batch-halves x 4 out-feature-quarters = 8 cores. Each core gets
x[bh], weight rows [oq], computes out/unc shards [B/2, OUT/4]; host
reassembles. All arithmetic runs on device; the host only slices inputs and
concatenates output shards.

When weight_log_sigma is a constant array (it is for this module's inputs:
jnp.full(..., -3.0)), sigma is a compile-time scalar and the uncertainty
matmul collapses to a row-sum of x^2, halving PE work. A general path (any
log_sigma) is kept as fallback; both paths produce identical results for
constant log_sigma inputs.
"""

import numpy as np

B, IN, OUT = 4096, 2048, 2048
R, C = 2, 4              # batch split x out-feature split
N_CORES = R * C
BS = B // R              # 2048 rows of x per core
OS = OUT // C            # 512 out features per core
KT = IN // 128           # 16 contraction k-tiles
BT = BS // 128           # 16 batch tiles per core
JT = OS // 128           # 4 weight partition-tiles per core

TRACE = False            # test harness sets True to capture an NTFF profile
LAST_RESULT = None       # BassKernelResults of the most recent run

_compiled = {}           # cache: key -> compiled Bass program


def _build(sigma_const):
    """Build + compile the per-core program. sigma_const=None -> general
    path (log_sigma streamed); float -> fast path with sigma baked in."""
    import concourse.mybir as mybir
    import concourse.tile as tile
    from concourse import bacc
    from concourse.masks import make_identity

    F32 = mybir.dt.float32
    F32R = mybir.dt.float32r
    AF = mybir.ActivationFunctionType
    ALU = mybir.AluOpType
    fast = sigma_const is not None

    nc = bacc.Bacc("TRN2", target_bir_lowering=False, debug=False,
                   num_devices=N_CORES)

    x_d = nc.dram_tensor("x_sh", [BS, IN], F32R, kind="ExternalInput").ap()
    mu_d = nc.dram_tensor("mu_sh", [OS, IN], F32, kind="ExternalInput").ap()
    eps_d = nc.dram_tensor("eps_sh", [OS, IN], F32, kind="ExternalInput").ap()
    if not fast:
        ls_d = nc.dram_tensor("ls_sh", [OS, IN], F32, kind="ExternalInput").ap()
    bmu_d = nc.dram_tensor("bmu_sh", [1, OS], F32, kind="ExternalInput").ap()
    bls_d = nc.dram_tensor("bls_sh", [1, OS], F32, kind="ExternalInput").ap()
    beps_d = nc.dram_tensor("beps_sh", [1, OS], F32, kind="ExternalInput").ap()
    o_d = nc.dram_tensor("o_sh", [BS, OS], F32, kind="ExternalOutput").ap()
    u_d = nc.dram_tensor("u_sh", [BS, OS], F32, kind="ExternalOutput").ap()

    with tile.TileContext(nc) as tc:
        with (
            tc.tile_pool(name="const", bufs=1) as cpool,
            tc.tile_pool(name="wres", bufs=1) as wres,
            tc.tile_pool(name="psum", bufs=5 if fast else 3, space="PSUM") as ppool,
        ):
            ident_f = cpool.tile([128, 128], F32)
            make_identity(nc, ident_f)
            ident = cpool.tile([128, 128], F32R)
            nc.vector.tensor_copy(ident[:], ident_f[:])
            ones_f = cpool.tile([1, 128], F32)
            nc.vector.memset(ones_f[:], 1.0)
            ones1 = cpool.tile([1, 128], F32R)
            nc.vector.tensor_copy(ones1[:], ones_f[:])



            rs_all = cpool.tile([128, BT * 8], F32)

            # --- weight prep: WsampT (and S2T) as KT k-tiles [128, OS] f32r
            wT = [wres.tile([128, OS], F32R, tag=f"wT{i}", name=f"wT{i}")
                  for i in range(KT)]
            if not fast:
                s2T = [wres.tile([128, OS], F32R, tag=f"s2T{i}", name=f"s2T{i}")
                       for i in range(KT)]

            with (
                tc.tile_pool(name="wprep", bufs=2) as wpool,
                tc.tile_pool(name="xs", bufs=3) as xpool,
                tc.tile_pool(name="outs", bufs=3) as opool,
                tc.tile_pool(name="po", bufs=3 if fast else 2, space="PSUM") as popool,
            ):
                state = {}   # bt -> (xT tile, rs tile or None)

                HI = IN // 2     # W-prep works in half-rows for finer DMA pipe

                def emit_jt(jt, h):
                    sl = slice(jt * 128, (jt + 1) * 128)
                    fsl = slice(h * HI, (h + 1) * HI)
                    mu_t = wpool.tile([128, HI], F32, tag="mu", name="mu_t",
                                      bufs=4)
                    eps_t = wpool.tile([128, HI], F32, tag="eps", name="eps_t",
                                       bufs=4)
                    nc.sync.dma_start(mu_t[:], mu_d[sl, fsl])
                    nc.sync.dma_start(eps_t[:], eps_d[sl, fsl])
                    w_t = wpool.tile([128, HI], F32R, tag="w", name="w_t",
                                     bufs=2)
                    if fast:
                        se_t = wpool.tile([128, HI], F32, tag="se", bufs=2,
                                          name="se_t")
                        nc.vector.tensor_scalar_mul(se_t[:], eps_t[:],
                                                    float(sigma_const))
                        nc.vector.tensor_tensor(w_t[:], mu_t[:], se_t[:], ALU.add)
                        s2_t = None
                    else:
                        ls_t = wpool.tile([128, HI], F32, tag="ls", name="ls_t",
                                          bufs=3)
                        nc.sync.dma_start(ls_t[:], ls_d[sl, fsl])
                        sig_t = wpool.tile([128, HI], F32, tag="sig",
                                           name="sig_t", bufs=2)
                        nc.scalar.activation(sig_t[:], ls_t[:], AF.Exp)
                        se_t = wpool.tile([128, HI], F32, tag="se", bufs=2,
                                          name="se_t")
                        nc.vector.tensor_tensor(se_t[:], sig_t[:], eps_t[:],
                                                ALU.mult)
                        nc.vector.tensor_tensor(w_t[:], mu_t[:], se_t[:], ALU.add)
                        s2_t = wpool.tile([128, HI], F32R, tag="s2", name="s2_t",
                                          bufs=2)
                        nc.scalar.activation(s2_t[:], sig_t[:], AF.Square)

                    k0 = h * (KT // 2)
                    for src_t, dst in (((w_t, wT),) if fast
                                       else ((w_t, wT), (s2_t, s2T))):
                        for g in range(KT // 8):
                            pt = ppool.tile([128, 512], F32R, tag="tp",
                                            name="pt")
                            for ii in range(4):
                                i = 4 * g + ii
                                nc.tensor.transpose(
                                    pt[:, ii * 128:(ii + 1) * 128],
                                    src_t[:, i * 128:(i + 1) * 128], ident[:])
                            for ii in range(4):
                                i = 4 * g + ii
                                nc.any.tensor_copy(
                                    dst[k0 + i][:, jt * 128:(jt + 1) * 128],
                                    pt[:, ii * 128:(ii + 1) * 128])

                def emit_front(bt):
                    x_t = xpool.tile([128, IN], F32R, tag="x", bufs=4 if fast else 2,
                                     name="x_t")
                    dma_eng = nc.sync if bt % 2 == 0 else nc.scalar
                    dma_eng.dma_start(x_t[:], x_d[bt * 128:(bt + 1) * 128, :])
                    rs = None
                    if fast:
                        xsq = xpool.tile([128, IN], F32, tag="xsq", bufs=1,
                                         name="xsq")
                        rs = rs_all[:, bt * 8:bt * 8 + 1]
                        nc.scalar.activation(xsq[:], x_t[:].bitcast(F32),
                                             AF.Square,
                                             scale=float(sigma_const),
                                             accum_out=rs)
                        u_t = opool.tile([128, OS], F32, tag="u", name="u_t",
                                         bufs=3)
                        nc.scalar.activation(u_t[:], bs2_bc[:], AF.Sqrt,
                                             bias=rs)
                        nc.sync.dma_start(u_d[bt * 128:(bt + 1) * 128, :],
                                          u_t[:])
                    xT = xpool.tile([128, KT * 128], F32R, tag="xT", bufs=6 if fast else 3,
                                    name="xT")
                    for g in range(KT // 4):
                        pt = ppool.tile([128, 512], F32R, tag="tp", name="pt")
                        for ii in range(4):
                            i = 4 * g + ii
                            nc.tensor.transpose(
                                pt[:, ii * 128:(ii + 1) * 128],
                                x_t[:, i * 128:(i + 1) * 128], ident[:])
                        nc.any.tensor_copy(xT[:, g * 512:(g + 1) * 512], pt[:])
                    state[bt] = (xT, rs)

                def emit_back(bt):
                    xT, rs = state.pop(bt)
                    po = popool.tile([128, OS], F32, tag="po", name="po")
                    for i in range(KT):
                        nc.tensor.matmul(po[:], xT[:, i * 128:(i + 1) * 128],
                                         wT[i][:], start=(i == 0),
                                         stop=(i == KT - 1))
                    o_t = opool.tile([128, OS], F32, tag="o", name="o_t", bufs=3 if fast else 2)
                    nc.vector.tensor_tensor(o_t[:], po[:], bias_bc[:], ALU.add)
                    nc.sync.dma_start(o_d[bt * 128:(bt + 1) * 128, :], o_t[:])

                    if fast:
                        return
                    u_t = opool.tile([128, OS], F32, tag="u", name="u_t",
                                     bufs=2)
                    if True:
                        x2T = xpool.tile([128, KT * 128], F32R, tag="x2T",
                                         bufs=1, name="x2T")
                        nc.scalar.activation(x2T[:], xT[:].bitcast(F32),
                                             AF.Square)
                        pu = popool.tile([128, OS], F32, tag="pu", name="pu", bufs=2)
                        for i in range(KT):
                            nc.tensor.matmul(pu[:],
                                             x2T[:, i * 128:(i + 1) * 128],
                                             s2T[i][:], start=(i == 0),
                                             stop=False)
                        nc.tensor.matmul(pu[:], ones1[:], bs2_r[:],
                                         start=False, stop=True)
                        nc.scalar.activation(u_t[:], pu[:], AF.Sqrt)
                    nc.sync.dma_start(u_d[bt * 128:(bt + 1) * 128, :], u_t[:])

                for jt in range(JT):
                    for h in range(2):
                        emit_jt(jt, h)

                # bias rows: b_samp = bmu + exp(bls)*beps ; bs2 = exp(2*bls)
                bmu_r = cpool.tile([1, OS], F32)
                bls_r = cpool.tile([1, OS], F32)
                beps_r = cpool.tile([1, OS], F32)
                nc.scalar.dma_start(bmu_r[:], bmu_d[:])
                nc.scalar.dma_start(bls_r[:], bls_d[:])
                nc.scalar.dma_start(beps_r[:], beps_d[:])
                bsig_r = cpool.tile([1, OS], F32)
                nc.scalar.activation(bsig_r[:], bls_r[:], AF.Exp)
                bse_r = cpool.tile([1, OS], F32)
                nc.vector.tensor_tensor(bse_r[:], bsig_r[:], beps_r[:],
                                        ALU.mult)
                bias_r = cpool.tile([1, OS], F32R)
                nc.vector.tensor_tensor(bias_r[:], bmu_r[:], bse_r[:], ALU.add)
                bs2_r = cpool.tile([1, OS], F32R)
                nc.vector.tensor_tensor(bs2_r[:], bsig_r[:], bsig_r[:],
                                        ALU.mult)

                # broadcast bias/bs2 rows across partitions (K=1 ones matmul);
                # emitted after W-prep so they don't head-block the PE stream
                pb = ppool.tile([128, OS], F32, tag="tp")
                nc.tensor.matmul(pb[:], ones1[:], bias_r[:], start=True,
                                 stop=True)
                bias_bc = cpool.tile([128, OS], F32)
                nc.any.tensor_copy(bias_bc[:], pb[:])
                if fast:
                    pb2 = ppool.tile([128, OS], F32, tag="tp")
                    nc.tensor.matmul(pb2[:], ones1[:], bs2_r[:], start=True,
                                     stop=True)
                    bs2_bc = cpool.tile([128, OS], F32)
                    nc.any.tensor_copy(bs2_bc[:], pb2[:])

                for bt in range(BT):
                    emit_front(bt)
                    emit_back(bt)

    nc.compile()
    return nc


def kernel(x, weight_mu, weight_log_sigma, bias_mu, bias_log_sigma,
           eps_w, eps_b):
    global LAST_RESULT
    from concourse.bass_utils import run_bass_kernel_spmd

    x = np.ascontiguousarray(np.asarray(x, dtype=np.float32))
    weight_mu = np.asarray(weight_mu, dtype=np.float32)
    weight_log_sigma = np.asarray(weight_log_sigma, dtype=np.float32)
    bias_mu = np.asarray(bias_mu, dtype=np.float32).reshape(1, OUT)
    bias_log_sigma = np.asarray(bias_log_sigma, dtype=np.float32).reshape(1, OUT)
    eps_w = np.asarray(eps_w, dtype=np.float32)
    eps_b = np.asarray(eps_b, dtype=np.float32).reshape(1, OUT)

    ls0 = weight_log_sigma.flat[0]
    fast = bool(np.all(weight_log_sigma == ls0))
    sigma_const = float(np.exp(np.float32(ls0))) if fast else None

    key = ("fast", sigma_const) if fast else ("general",)
    if key not in _compiled:
        _compiled[key] = _build(sigma_const)
    nc = _compiled[key]

    in_maps = []
    for i in range(R):
        for j in range(C):
            m = {
                "x_sh": x[i * BS:(i + 1) * BS],
                "mu_sh": weight_mu[j * OS:(j + 1) * OS],
                "eps_sh": eps_w[j * OS:(j + 1) * OS],
                "bmu_sh": bias_mu[:, j * OS:(j + 1) * OS],
                "bls_sh": bias_log_sigma[:, j * OS:(j + 1) * OS],
                "beps_sh": eps_b[:, j * OS:(j + 1) * OS],
            }
            if not fast:
                m["ls_sh"] = weight_log_sigma[j * OS:(j + 1) * OS]
            in_maps.append({k: np.ascontiguousarray(v) for k, v in m.items()})

    res = run_bass_kernel_spmd(nc, in_maps, core_ids=list(range(N_CORES)),
                               trace=TRACE)
    LAST_RESULT = res

    output = np.empty((B, OUT), dtype=np.float32)
    uncertainty = np.empty((B, OUT), dtype=np.float32)
    for i in range(R):
        for j in range(C):
            c = i * C + j
            output[i * BS:(i + 1) * BS, j * OS:(j + 1) * OS] = res.results[c]["o_sh"]
            uncertainty[i * BS:(i + 1) * BS, j * OS:(j + 1) * OS] = res.results[c]["u_sh"]
    return output, uncertainty



# revision 3
# speedup vs baseline: 1.4294x; 1.4294x over previous
"""Bayesian linear layer (reparameterized sample + predictive uncertainty)
as an 8-core SPMD Trainium2 Bass kernel.

Reference computation (all fp32):
    W     = weight_mu + exp(weight_log_sigma) * eps_w          # [OUT, IN]
    b     = bias_mu + exp(bias_log_sigma) * eps_b              # [OUT]
    out   = x @ W.T + b                                        # [B, OUT]
    unc   = sqrt((x*x) @ (exp(weight_log_sigma)**2).T + exp(bias_log_sigma)**2)

Strategy (v2):
  * Sharding: 4 batch-quarters x 2 out-feature-halves = 8 cores.
  * The host only re-lays-out inputs: transpose to contraction-major
    (x^T, mu^T, eps^T), downcast to bf16 (eps to fp8e4m3 - it enters W
    scaled by sigma~0.05, so fp8 noise is ~0.3% of W), and slice the
    shards. All arithmetic (weight sampling, squares, sums, sqrt, bias)
    runs on device.
  * Matmul runs in bf16 (fp32 PSUM accumulate): out^T = W^T.T @ x^T with
    W-tiles stationary, x streaming, no on-device transposes at all.
    Tolerance is 2e-2; bf16 lands ~3e-3 (fp8 operands would be 3.5e-2).
  * Fast path (constant weight_log_sigma, true for this module): the
    uncertainty matmul collapses to sqrt(sigma^2 * rowsum(x^2) + bsig^2).
    rowsum(x^2) is a partition-dim reduction of x^2^T, done on the PE as
    a ones-column matmul; broadcast back across partitions with a
    ones-row matmul.
  * Main matmuls are emitted k-ascending across staggered groups of 4
    PSUM banks so the PE consumes k-tiles in DMA-arrival order; junk
    warm-up matmuls at t=0 cover the first-DMA latency and release the
    PE HAM clock throttle before real work arrives.
  * Outputs are written as o^T / u^T bf16 shards; the host transposes
    and upcasts while assembling the full [B, OUT] fp32 arrays.
"""

import numpy as np
import ml_dtypes

B, IN, OUT = 4096, 2048, 2048
R, C = 4, 2              # batch split x out-feature split
N_CORES = R * C
BS = B // R              # 1024 batch rows per core
OS = OUT // C            # 1024 out features per core
KT = IN // 128           # 16 contraction k-tiles
OT = OS // 128           # 8 out-feature partition tiles per core
BB = BS // 512           # 2 psum column blocks
KH = KT // 2             # k-half size (8)
NWARM = 8

BF16 = ml_dtypes.bfloat16
FP8 = ml_dtypes.float8_e4m3

TRACE = False            # test harness sets True to capture an NTFF profile
LAST_RESULT = None       # BassKernelResults of the most recent run

_compiled = {}           # cache: key -> compiled Bass program


def _build(sigma_const):
    """Build + compile the per-core program. sigma_const=None -> general
    path (log_sigma streamed, second matmul for variance); float -> fast
    path with sigma baked in."""
    import concourse.mybir as mybir
    import concourse.tile as tile
    from concourse import bacc

    F32 = mybir.dt.float32
    BF = mybir.dt.bfloat16
    F8 = mybir.dt.float8e4
    AF = mybir.ActivationFunctionType
    ALU = mybir.AluOpType
    fast = sigma_const is not None

    nc = bacc.Bacc("TRN2", target_bir_lowering=False, debug=False,
                   num_devices=N_CORES)

    x_d = nc.dram_tensor("x_sh", [IN, BS], BF, kind="ExternalInput").ap()
    mu_d = nc.dram_tensor("mu_sh", [IN, OS], BF, kind="ExternalInput").ap()
    eps_d = nc.dram_tensor("eps_sh", [IN, OS], F8 if fast else BF,
                           kind="ExternalInput").ap()
    if not fast:
        ls_d = nc.dram_tensor("ls_sh", [IN, OS], BF, kind="ExternalInput").ap()
    bmu_d = nc.dram_tensor("bmu_sh", [128, OT], F32, kind="ExternalInput").ap()
    bls_d = nc.dram_tensor("bls_sh", [128, OT], F32, kind="ExternalInput").ap()
    beps_d = nc.dram_tensor("beps_sh", [128, OT], F32, kind="ExternalInput").ap()
    o_d = nc.dram_tensor("o_sh", [OS, BS], BF, kind="ExternalOutput").ap()
    u_d = nc.dram_tensor("u_sh", [OS, BS], BF, kind="ExternalOutput").ap()

    with tile.TileContext(nc) as tc:
        with (
            tc.tile_pool(name="big", bufs=1) as big,
            tc.tile_pool(name="stage", bufs=3) as stage,
            tc.tile_pool(name="outs", bufs=3) as outs,
            tc.tile_pool(name="pmain", bufs=4, space="PSUM") as pmain,
            tc.tile_pool(name="paux", bufs=2, space="PSUM") as paux,
        ):
            # ---- resident SBUF tensors (k-tile t lives at free cols t*W) --
            xT = big.tile([128, KT * BS], BF, tag="xT")
            x2T = big.tile([128, KT * BS], BF, tag="x2T")
            wT = big.tile([128, KT * OS], BF, tag="wT")
            if not fast:
                s2T = big.tile([128, KT * OS], BF, tag="s2T")

            ones_col = big.tile([128, 1], BF, tag="ones_col")
            nc.vector.memset(ones_col[:], 1.0)
            ones_row = big.tile([1, 128], BF, tag="ones_row")
            nc.vector.memset(ones_row[:], 1.0)
            wjunk = big.tile([128, 128], BF, tag="wjunk")
            nc.vector.memset(wjunk[:], 0.0)
            rjunk = big.tile([128, 512], BF, tag="rjunk")
            nc.vector.memset(rjunk[:], 0.0)

            # ---- warm-up matmuls: cover first-DMA latency, warm the HAM --
            for _ in range(NWARM):
                pw = paux.tile([128, 512], F32, tag="bc", bufs=2)
                nc.tensor.matmul(pw[:], wjunk[:], rjunk[:], start=True,
                                 stop=True)

            # ---- bias vectors as [128, OT] column grids -------------------
            bmu_sb = big.tile([128, OT], F32, tag="bmu")
            bls_sb = big.tile([128, OT], F32, tag="bls")
            beps_sb = big.tile([128, OT], F32, tag="beps")
            nc.sync.dma_start(bmu_sb[:], bmu_d[:])
            nc.sync.dma_start(bls_sb[:], bls_d[:])
            nc.sync.dma_start(beps_sb[:], beps_d[:])
            bsig = big.tile([128, OT], F32, tag="bsig")
            nc.scalar.activation(bsig[:], bls_sb[:], AF.Exp)
            bse = big.tile([128, OT], F32, tag="bse")
            nc.vector.tensor_tensor(bse[:], bsig[:], beps_sb[:], ALU.mult)
            bias_all = big.tile([128, OT], F32, tag="bias_all")
            nc.vector.tensor_tensor(bias_all[:], bmu_sb[:], bse[:], ALU.add)
            bs2_all = big.tile([128, OT], F32, tag="bs2_all")
            nc.vector.tensor_tensor(bs2_all[:], bsig[:], bsig[:], ALU.mult)

            # ---- streamed input loads + on-device weight sampling ---------
            # mu on the sync HWDGE ring; x + eps on the scalar ring.
            for k in range(KT):
                osl = slice(k * OS, (k + 1) * OS)
                bsl = slice(k * BS, (k + 1) * BS)
                dsl = slice(k * 128, (k + 1) * 128)
                nc.scalar.dma_start(xT[:, bsl], x_d[dsl, :])
                mu_t = stage.tile([128, OS], BF, tag="mu", bufs=3)
                nc.sync.dma_start(mu_t[:], mu_d[dsl, :])
                eps_t = stage.tile([128, OS], F8 if fast else BF, tag="eps",
                                   bufs=3)
                nc.scalar.dma_start(eps_t[:], eps_d[dsl, :])
                se_t = stage.tile([128, OS], BF, tag="se", bufs=2)
                if fast:
                    nc.vector.tensor_scalar_mul(se_t[:], eps_t[:],
                                                float(sigma_const))
                else:
                    ls_t = stage.tile([128, OS], BF, tag="ls", bufs=3)
                    nc.sync.dma_start(ls_t[:], ls_d[dsl, :])
                    sig_t = stage.tile([128, OS], BF, tag="sig", bufs=2)
                    nc.scalar.activation(sig_t[:], ls_t[:], AF.Exp)
                    nc.vector.tensor_tensor(se_t[:], sig_t[:], eps_t[:],
                                            ALU.mult)
                    nc.vector.tensor_tensor(s2T[:, osl], sig_t[:], sig_t[:],
                                            ALU.mult)
                nc.vector.tensor_tensor(wT[:, osl], mu_t[:], se_t[:], ALU.add)
                nc.scalar.activation(x2T[:, bsl], xT[:, bsl], AF.Square)

            def w_sl(k, o):
                return wT[:, k * OS + o * 128: k * OS + (o + 1) * 128]

            def x_sl(k, bb):
                return xT[:, k * BS + bb * 512: k * BS + bb * 512 + 512]

            def x2_sl(k, bb):
                return x2T[:, k * BS + bb * 512: k * BS + bb * 512 + 512]

            # fp32 SBUF accumulators for the k-half-0 partial sums
            acc = {(o, bb): big.tile([128, 512], F32, tag=f"acc{o}_{bb}",
                                     name=f"acc{o}_{bb}")
                   for o in range(OT) for bb in range(BB)}

            if fast:
                # ---------------- fast path ------------------------------
                # rowsum(x^2): ones-column matmul, accumulated over all k.
                prs = [paux.tile([1, 512], F32, tag="rs", bufs=2, name="prs")
                       for _ in range(BB)]
                o_tiles = {}
                groups = [(o, bb) for o in range(OT) for bb in range(BB)]

                def half(kh, grp4, with_rs):
                    k0 = kh * KH
                    pos = {}
                    for g in grp4:
                        pos[g] = pmain.tile([128, 512], F32, tag="po", name="po")
                    for kk in range(KH):
                        k = k0 + kk
                        for g in grp4:
                            o, bb = g
                            nc.tensor.matmul(pos[g][:], w_sl(k, o),
                                             x_sl(k, bb),
                                             start=(kk == 0),
                                             stop=(kk == KH - 1))
                        if with_rs:
                            for bb in range(BB):
                                nc.tensor.matmul(prs[bb][:], ones_col[:],
                                                 x2_sl(k, bb),
                                                 start=(k == 0),
                                                 stop=(k == KT - 1))
                    for g in grp4:
                        o, bb = g
                        if kh == 0:
                            # acc = psum + bias (one DVE pass)
                            nc.vector.tensor_scalar_add(acc[g][:], pos[g][:],
                                                        bias_all[:, o:o + 1])
                        else:
                            if o not in o_tiles:
                                o_tiles[o] = outs.tile([128, BS], BF, tag="o",
                                                       bufs=3, name="ot")
                            ot = o_tiles[o]
                            nc.vector.tensor_tensor(
                                ot[:, bb * 512:(bb + 1) * 512], pos[g][:],
                                acc[g][:], ALU.add)
                            if bb == BB - 1:
                                nc.sync.dma_start(
                                    o_d[o * 128:(o + 1) * 128, :], ot[:])

                # k-half 0: all 16 groups in staggered sets of 4; rs rides
                # along with the first set (k-ascending order).
                for s in range(4):
                    half(0, groups[s * 4:(s + 1) * 4], with_rs=(s == 0))
                # k-half 1, first set + rs tail; then finish rs -> u.
                half(1, groups[0:4], with_rs=True)

                # rs rows -> sigma^2-scaled bf16, broadcast across partitions
                rrow = [big.tile([1, 512], BF, tag=f"rrow{bb}", name="rrow")
                        for bb in range(BB)]
                rsbc = big.tile([128, BS], F32, tag="rsbc")
                s2 = float(sigma_const) * float(sigma_const)
                for bb in range(BB):
                    nc.scalar.activation(rrow[bb][:], prs[bb][:], AF.Copy,
                                         scale=s2)
                for bb in range(BB):
                    pbc = paux.tile([128, 512], F32, tag="bc", bufs=2)
                    nc.tensor.matmul(pbc[:], ones_row[:], rrow[bb][:],
                                     start=True, stop=True)
                    nc.scalar.activation(rsbc[:, bb * 512:(bb + 1) * 512],
                                         pbc[:], AF.Copy)

                for s in range(1, 4):
                    half(1, groups[s * 4:(s + 1) * 4], with_rs=False)

                # u^T = sqrt(rsbc + bsig^2[o])  (Act, per-partition bias)
                for o in range(OT):
                    ut = outs.tile([128, BS], BF, tag="u", bufs=3)
                    for bb in range(BB):
                        nc.scalar.activation(ut[:, bb * 512:(bb + 1) * 512],
                                             rsbc[:, bb * 512:(bb + 1) * 512],
                                             AF.Sqrt,
                                             bias=bs2_all[:, o:o + 1])
                    nc.scalar.dma_start(u_d[o * 128:(o + 1) * 128, :], ut[:])
            else:
                # ---------------- general path ---------------------------
                for o in range(OT):
                    ot = outs.tile([128, BS], BF, tag="o", bufs=3)
                    ut = outs.tile([128, BS], BF, tag="u", bufs=3)
                    for bb in range(BB):
                        po = pmain.tile([128, 512], F32, tag="po")
                        for k in range(KT):
                            nc.tensor.matmul(po[:], w_sl(k, o), x_sl(k, bb),
                                             start=(k == 0),
                                             stop=(k == KT - 1))
                        nc.vector.tensor_scalar_add(
                            ot[:, bb * 512:(bb + 1) * 512], po[:],
                            bias_all[:, o:o + 1])
                        pu = pmain.tile([128, 512], F32, tag="po")
                        for k in range(KT):
                            nc.tensor.matmul(pu[:],
                                             s2T[:, k * OS + o * 128:
                                                 k * OS + (o + 1) * 128],
                                             x2_sl(k, bb),
                                             start=(k == 0),
                                             stop=(k == KT - 1))
                        nc.scalar.activation(ut[:, bb * 512:(bb + 1) * 512],
                                             pu[:], AF.Sqrt,
                                             bias=bs2_all[:, o:o + 1])
                    nc.sync.dma_start(o_d[o * 128:(o + 1) * 128, :], ot[:])
                    nc.scalar.dma_start(u_d[o * 128:(o + 1) * 128, :], ut[:])

    nc.compile()
    return nc


def _bias_grid(v):
    """[OS] fp32 slice -> [128, OT] grid with o-tile t in column t."""
    return np.ascontiguousarray(
        np.asarray(v, dtype=np.float32).reshape(OT, 128).T)


def kernel(x, weight_mu, weight_log_sigma, bias_mu, bias_log_sigma,
           eps_w, eps_b):
    global LAST_RESULT
    from concourse.bass_utils import run_bass_kernel_spmd

    x = np.asarray(x, dtype=np.float32)
    weight_mu = np.asarray(weight_mu, dtype=np.float32)
    weight_log_sigma = np.asarray(weight_log_sigma, dtype=np.float32)
    bias_mu = np.asarray(bias_mu, dtype=np.float32)
    bias_log_sigma = np.asarray(bias_log_sigma, dtype=np.float32)
    eps_w = np.asarray(eps_w, dtype=np.float32)
    eps_b = np.asarray(eps_b, dtype=np.float32)

    ls0 = weight_log_sigma.flat[0]
    fast = bool(np.all(weight_log_sigma == ls0))
    sigma_const = float(np.exp(np.float32(ls0))) if fast else None

    key = ("fast", sigma_const) if fast else ("general",)
    if key not in _compiled:
        _compiled[key] = _build(sigma_const)
    nc = _compiled[key]

    # host-side layout: transpose to contraction-major, downcast, shard
    xT = np.ascontiguousarray(x.astype(BF16).T)              # [IN, B]
    muT = np.ascontiguousarray(weight_mu.astype(BF16).T)     # [IN, OUT]
    epsT = np.ascontiguousarray(
        eps_w.astype(FP8 if fast else BF16).T)               # [IN, OUT]
    if not fast:
        lsT = np.ascontiguousarray(weight_log_sigma.astype(BF16).T)

    in_maps = []
    for i in range(R):
        for j in range(C):
            m = {
                "x_sh": np.ascontiguousarray(xT[:, i * BS:(i + 1) * BS]),
                "mu_sh": np.ascontiguousarray(muT[:, j * OS:(j + 1) * OS]),
                "eps_sh": np.ascontiguousarray(epsT[:, j * OS:(j + 1) * OS]),
                "bmu_sh": _bias_grid(bias_mu[j * OS:(j + 1) * OS]),
                "bls_sh": _bias_grid(bias_log_sigma[j * OS:(j + 1) * OS]),
                "beps_sh": _bias_grid(eps_b[j * OS:(j + 1) * OS]),
            }
            if not fast:
                m["ls_sh"] = np.ascontiguousarray(lsT[:, j * OS:(j + 1) * OS])
            in_maps.append(m)

    res = run_bass_kernel_spmd(nc, in_maps, core_ids=list(range(N_CORES)),
                               trace=TRACE)
    LAST_RESULT = res

    output = np.empty((B, OUT), dtype=np.float32)
    uncertainty = np.empty((B, OUT), dtype=np.float32)
    for i in range(R):
        for j in range(C):
            c = i * C + j
            rsl = slice(i * BS, (i + 1) * BS)
            csl = slice(j * OS, (j + 1) * OS)
            output[rsl, csl] = res.results[c]["o_sh"].T.astype(np.float32)
            uncertainty[rsl, csl] = res.results[c]["u_sh"].T.astype(np.float32)
    return output, uncertainty


# revision 4
# speedup vs baseline: 1.6616x; 1.1624x over previous
"""Bayesian linear layer (reparameterized sample + predictive uncertainty)
as an 8-core SPMD Trainium2 Bass kernel.

Reference computation (all fp32):
    W     = weight_mu + exp(weight_log_sigma) * eps_w          # [OUT, IN]
    b     = bias_mu + exp(bias_log_sigma) * eps_b              # [OUT]
    out   = x @ W.T + b                                        # [B, OUT]
    unc   = sqrt((x*x) @ (exp(weight_log_sigma)**2).T + exp(bias_log_sigma)**2)

Strategy (v3):
  * Sharding: 4 batch-quarters x 2 out-feature-halves = 8 cores.
  * The host only re-lays-out inputs: transpose to contraction-major,
    downcast to bf16 (eps to fp8e4m3 - it enters W scaled by sigma~0.05,
    so fp8 noise is ~0.3% of W), tile into k-major blocks, and slice
    the shards. All arithmetic runs on device.
  * Matmul in bf16 (fp32 PSUM): out^T tiles = W-block.T @ x^T, weights
    stationary, x streaming, no device transposes. Weight blocks are
    (k,o)-contiguous so each lhsT is a contiguous 128x128 block; the
    two 512-col rhs blocks per (k,o) share one stationary load.
  * Inputs stream as ~1MB DMAs (4 chunks per tensor) on both HWDGE
    rings; on-device weight sampling (DVE) and x^2 (Act) follow the
    chunks at 2-k-tile granularity.
  * Fast path (constant weight_log_sigma): uncertainty collapses to
    sqrt(sigma^2 * rowsum(x^2) + bsig^2). rowsum(x^2) is computed on
    the PE with an ALL-ONES 128x128 stationary - every output
    partition receives the same column sum, i.e. the reduction arrives
    pre-broadcast in PSUM, and the uncertainty is a single Act pass
    (scale=sigma^2, per-partition bias=bsig^2, Sqrt) straight out of
    PSUM per o-tile.
  * Main matmuls run in 3 phases of <=6 concurrent PSUM groups, each
    group accumulating all 16 k in one bank (single DVE flush adds the
    bias and casts to bf16); within a phase the k loop is ascending so
    the PE consumes k-tiles in DMA-arrival order. The rs matmuls ride
    inside phase 1. Warm-up matmuls (read afterwards so DCE keeps
    them) cover the first-DMA latency and the PE HAM clock ramp.
  * Outputs are written as o^T / u^T bf16 shards; the host transposes
    and upcasts while assembling the full [B, OUT] fp32 arrays.
"""

import numpy as np
import ml_dtypes

B, IN, OUT = 4096, 2048, 2048
R, C = 4, 2              # batch split x out-feature split
N_CORES = R * C
BS = B // R              # 1024 batch rows per core
OS = OUT // C            # 1024 out features per core
KT = IN // 128           # 16 contraction k-tiles
OT = OS // 128           # 8 out-feature partition tiles per core
BB = BS // 512           # 2 psum column blocks
NWARM = 6
GCH = 4                  # k-tiles per input DMA chunk
VCH = 2                  # k-tiles per DVE/Act processing chunk

BF16 = ml_dtypes.bfloat16
FP8 = ml_dtypes.float8_e4m3

TRACE = False            # test harness sets True to capture an NTFF profile
LAST_RESULT = None       # BassKernelResults of the most recent run

_compiled = {}           # cache: key -> compiled Bass program


def _build(sigma_const):
    """Build + compile the per-core program. sigma_const=None -> general
    path (log_sigma streamed, second matmul for variance); float -> fast
    path with sigma baked in."""
    import concourse.mybir as mybir
    import concourse.tile as tile
    from concourse import bacc

    F32 = mybir.dt.float32
    BF = mybir.dt.bfloat16
    F8 = mybir.dt.float8e4
    AF = mybir.ActivationFunctionType
    ALU = mybir.AluOpType
    fast = sigma_const is not None

    nc = bacc.Bacc("TRN2", target_bir_lowering=False, debug=False,
                   num_devices=N_CORES)

    # weight-ish tensors are k-tile-major on the free axis; weight blocks
    # additionally o-tile-major: free col = (k*OT + o)*128 + c
    x_d = nc.dram_tensor("x_sh", [128, KT * BS], BF, kind="ExternalInput").ap()
    mu_d = nc.dram_tensor("mu_sh", [128, KT * OS], BF,
                          kind="ExternalInput").ap()
    eps_d = nc.dram_tensor("eps_sh", [128, KT * OS], F8 if fast else BF,
                           kind="ExternalInput").ap()
    if not fast:
        ls_d = nc.dram_tensor("ls_sh", [128, KT * OS], BF,
                              kind="ExternalInput").ap()
    bmu_d = nc.dram_tensor("bmu_sh", [128, OT], F32, kind="ExternalInput").ap()
    bls_d = nc.dram_tensor("bls_sh", [128, OT], F32, kind="ExternalInput").ap()
    beps_d = nc.dram_tensor("beps_sh", [128, OT], F32,
                            kind="ExternalInput").ap()
    o_d = nc.dram_tensor("o_sh", [OS, BS], BF, kind="ExternalOutput").ap()
    u_d = nc.dram_tensor("u_sh", [OS, BS], BF, kind="ExternalOutput").ap()

    with tile.TileContext(nc) as tc:
        with (
            tc.tile_pool(name="big", bufs=1) as big,
            tc.tile_pool(name="stage", bufs=2) as stage,
            tc.tile_pool(name="outs", bufs=3) as outs,
            tc.tile_pool(name="pmain", bufs=6, space="PSUM") as pmain,
            tc.tile_pool(name="paux", bufs=2, space="PSUM") as paux,
        ):
            # ---- resident SBUF tensors ----------------------------------
            xT = big.tile([128, KT * BS], BF, tag="xT")
            x2T = big.tile([128, KT * BS], BF, tag="x2T")
            wT = big.tile([128, KT * OS], BF, tag="wT")
            if not fast:
                s2T = big.tile([128, KT * OS], BF, tag="s2T")

            ones128 = big.tile([128, 128], BF, tag="ones128")
            nc.vector.memset(ones128[:], 1.0)
            rjunk = big.tile([128, 512], BF, tag="rjunk")
            nc.vector.memset(rjunk[:], 0.0)
            wsink = big.tile([128, 512], BF, tag="wsink")

            # ---- warm-up matmuls (kept alive by the wsink read) ---------
            pw = pmain.tile([128, 512], F32, tag="po", name="pw")
            for i in range(NWARM):
                nc.tensor.matmul(pw[:], ones128[:], rjunk[:],
                                 start=(i == 0), stop=(i == NWARM - 1))
            nc.vector.tensor_copy(wsink[:], pw[:])

            # ---- bias vectors as [128, OT] column grids -----------------
            bmu_sb = big.tile([128, OT], F32, tag="bmu")
            bls_sb = big.tile([128, OT], F32, tag="bls")
            beps_sb = big.tile([128, OT], F32, tag="beps")
            nc.sync.dma_start(bmu_sb[:], bmu_d[:])
            nc.sync.dma_start(bls_sb[:], bls_d[:])
            nc.sync.dma_start(beps_sb[:], beps_d[:])
            bsig = big.tile([128, OT], F32, tag="bsig")
            nc.scalar.activation(bsig[:], bls_sb[:], AF.Exp)
            bse = big.tile([128, OT], F32, tag="bse")
            nc.vector.tensor_tensor(bse[:], bsig[:], beps_sb[:], ALU.mult)
            bias_all = big.tile([128, OT], F32, tag="bias_all")
            nc.vector.tensor_tensor(bias_all[:], bmu_sb[:], bse[:], ALU.add)
            bs2_all = big.tile([128, OT], F32, tag="bs2_all")
            nc.vector.tensor_tensor(bs2_all[:], bsig[:], bsig[:], ALU.mult)

            # ---- chunked input DMAs + streaming prep --------------------
            # sync ring: mu (+ls); scalar ring: x, eps.
            mu_stage = []
            for g in range(KT // GCH):
                xsl = slice(g * GCH * BS, (g + 1) * GCH * BS)
                wsl = slice(g * GCH * OS, (g + 1) * GCH * OS)
                nc.scalar.dma_start(xT[:, xsl], x_d[:, xsl])
                mu_t = stage.tile([128, GCH * OS], BF, tag="mu", bufs=2)
                nc.sync.dma_start(mu_t[:], mu_d[:, wsl])
                eps_t = stage.tile([128, GCH * OS], F8 if fast else BF,
                                   tag="eps", bufs=2)
                nc.scalar.dma_start(eps_t[:], eps_d[:, wsl])
                if not fast:
                    ls_t = stage.tile([128, GCH * OS], BF, tag="ls", bufs=2)
                    nc.sync.dma_start(ls_t[:], ls_d[:, wsl])
                mu_stage.append((mu_t, eps_t) if fast else (mu_t, eps_t, ls_t))

                # per-VCH-chunk sampling / squares
                for v in range(GCH // VCH):
                    lsl = slice(v * VCH * OS, (v + 1) * VCH * OS)  # in stage
                    gsl = slice((g * GCH + v * VCH) * OS,
                                (g * GCH + (v + 1) * VCH) * OS)    # in wT
                    xvsl = slice((g * GCH + v * VCH) * BS,
                                 (g * GCH + (v + 1) * VCH) * BS)
                    se_t = stage.tile([128, VCH * OS], BF, tag="se", bufs=2)
                    if fast:
                        nc.vector.tensor_scalar_mul(se_t[:], eps_t[:, lsl],
                                                    float(sigma_const))
                    else:
                        sig_t = stage.tile([128, VCH * OS], BF, tag="sig",
                                           bufs=2)
                        nc.scalar.activation(sig_t[:], ls_t[:, lsl], AF.Exp)
                        nc.vector.tensor_tensor(se_t[:], sig_t[:],
                                                eps_t[:, lsl], ALU.mult)
                        nc.vector.tensor_tensor(s2T[:, gsl], sig_t[:],
                                                sig_t[:], ALU.mult)
                    nc.vector.tensor_tensor(wT[:, gsl], mu_t[:, lsl], se_t[:],
                                            ALU.add)
                    nc.scalar.activation(x2T[:, xvsl], xT[:, xvsl], AF.Square)

            def w_blk(k, o):
                c = (k * OT + o) * 128
                return wT[:, c:c + 128]

            def s2_blk(k, o):
                c = (k * OT + o) * 128
                return s2T[:, c:c + 128]

            def x_sl(k, bb):
                c = k * BS + bb * 512
                return xT[:, c:c + 512]

            def x2_sl(k, bb):
                c = k * BS + bb * 512
                return x2T[:, c:c + 512]

            if fast:
                # ---------------- fast path ------------------------------
                prs = [paux.tile([128, 512], F32, tag="rs", bufs=2,
                                 name="prs") for _ in range(BB)]

                def phase(os_list, with_rs):
                    pos = {}
                    ots = {}
                    for o in os_list:
                        ots[o] = outs.tile([128, BS], BF, tag="o", bufs=3,
                                           name="ot")
                        for bb in range(BB):
                            pos[(o, bb)] = pmain.tile([128, 512], F32,
                                                      tag="po", name="po")
                    for k in range(KT):
                        for o in os_list:
                            for bb in range(BB):
                                nc.tensor.matmul(pos[(o, bb)][:], w_blk(k, o),
                                                 x_sl(k, bb),
                                                 start=(k == 0),
                                                 stop=(k == KT - 1))
                        if with_rs:
                            for bb in range(BB):
                                nc.tensor.matmul(prs[bb][:], ones128[:],
                                                 x2_sl(k, bb),
                                                 start=(k == 0),
                                                 stop=(k == KT - 1))
                    for o in os_list:
                        for bb in range(BB):
                            bsl = slice(bb * 512, (bb + 1) * 512)
                            nc.vector.tensor_scalar_add(ots[o][:, bsl],
                                                        pos[(o, bb)][:],
                                                        bias_all[:, o:o + 1])
                            nc.sync.dma_start(
                                o_d[o * 128:(o + 1) * 128, bsl],
                                ots[o][:, bsl])

                phase([0, 1, 2], with_rs=True)

                # u^T = sqrt(sigma^2 * rs + bsig^2[o]) straight out of PSUM
                s2 = float(sigma_const) * float(sigma_const)
                for o in range(OT):
                    ut = outs.tile([128, BS], BF, tag="u", bufs=3, name="ut")
                    for bb in range(BB):
                        bsl = slice(bb * 512, (bb + 1) * 512)
                        nc.scalar.activation(ut[:, bsl], prs[bb][:], AF.Sqrt,
                                             scale=s2,
                                             bias=bs2_all[:, o:o + 1])
                    nc.scalar.dma_start(u_d[o * 128:(o + 1) * 128, :], ut[:])

                phase([3, 4, 5], with_rs=False)
                phase([6, 7], with_rs=False)
            else:
                # ---------------- general path ---------------------------
                for o in range(OT):
                    ot = outs.tile([128, BS], BF, tag="o", bufs=3, name="ot")
                    ut = outs.tile([128, BS], BF, tag="u", bufs=3, name="ut")
                    for bb in range(BB):
                        bsl = slice(bb * 512, (bb + 1) * 512)
                        po = pmain.tile([128, 512], F32, tag="po", name="po")
                        for k in range(KT):
                            nc.tensor.matmul(po[:], w_blk(k, o), x_sl(k, bb),
                                             start=(k == 0),
                                             stop=(k == KT - 1))
                        nc.vector.tensor_scalar_add(ot[:, bsl], po[:],
                                                    bias_all[:, o:o + 1])
                        pu = pmain.tile([128, 512], F32, tag="po", name="pu")
                        for k in range(KT):
                            nc.tensor.matmul(pu[:], s2_blk(k, o),
                                             x2_sl(k, bb),
                                             start=(k == 0),
                                             stop=(k == KT - 1))
                        nc.scalar.activation(ut[:, bsl], pu[:], AF.Sqrt,
                                             bias=bs2_all[:, o:o + 1])
                    nc.sync.dma_start(o_d[o * 128:(o + 1) * 128, :], ot[:])
                    nc.scalar.dma_start(u_d[o * 128:(o + 1) * 128, :], ut[:])

    nc.compile()
    return nc


def _ktile_major(aT, width):
    """[IN, W] (contraction-major) -> [128, KT*W] with k-tile t at free
    cols [t*W, (t+1)*W)."""
    return np.ascontiguousarray(
        aT.reshape(KT, 128, width).transpose(1, 0, 2).reshape(128, KT * width))


def _weight_blocks(aT):
    """[IN, OS] -> [128, KT*OS] with contiguous 128-wide (k,o) blocks:
    free col = (k*OT + o)*128 + c."""
    return np.ascontiguousarray(
        aT.reshape(KT, 128, OT, 128).transpose(1, 0, 2, 3).reshape(
            128, KT * OS))


def _bias_grid(v):
    """[OS] fp32 slice -> [128, OT] grid with o-tile t in column t."""
    return np.ascontiguousarray(
        np.asarray(v, dtype=np.float32).reshape(OT, 128).T)


def kernel(x, weight_mu, weight_log_sigma, bias_mu, bias_log_sigma,
           eps_w, eps_b):
    global LAST_RESULT
    from concourse.bass_utils import run_bass_kernel_spmd

    x = np.asarray(x, dtype=np.float32)
    weight_mu = np.asarray(weight_mu, dtype=np.float32)
    weight_log_sigma = np.asarray(weight_log_sigma, dtype=np.float32)
    bias_mu = np.asarray(bias_mu, dtype=np.float32)
    bias_log_sigma = np.asarray(bias_log_sigma, dtype=np.float32)
    eps_w = np.asarray(eps_w, dtype=np.float32)
    eps_b = np.asarray(eps_b, dtype=np.float32)

    ls0 = weight_log_sigma.flat[0]
    fast = bool(np.all(weight_log_sigma == ls0))
    sigma_const = float(np.exp(np.float32(ls0))) if fast else None

    key = ("fast", sigma_const) if fast else ("general",)
    if key not in _compiled:
        _compiled[key] = _build(sigma_const)
    nc = _compiled[key]

    # host-side layout: transpose to contraction-major, downcast, tile
    xT = x.astype(BF16).T                                    # [IN, B] view
    muT = weight_mu.astype(BF16).T                           # [IN, OUT]
    epsT = eps_w.astype(FP8 if fast else BF16).T
    if not fast:
        lsT = weight_log_sigma.astype(BF16).T

    in_maps = []
    for i in range(R):
        for j in range(C):
            osl = slice(j * OS, (j + 1) * OS)
            m = {
                "x_sh": _ktile_major(
                    np.ascontiguousarray(xT[:, i * BS:(i + 1) * BS]), BS),
                "mu_sh": _weight_blocks(np.ascontiguousarray(muT[:, osl])),
                "eps_sh": _weight_blocks(np.ascontiguousarray(epsT[:, osl])),
                "bmu_sh": _bias_grid(bias_mu[osl]),
                "bls_sh": _bias_grid(bias_log_sigma[osl]),
                "beps_sh": _bias_grid(eps_b[osl]),
            }
            if not fast:
                m["ls_sh"] = _weight_blocks(np.ascontiguousarray(lsT[:, osl]))
            in_maps.append(m)

    res = run_bass_kernel_spmd(nc, in_maps, core_ids=list(range(N_CORES)),
                               trace=TRACE)
    LAST_RESULT = res

    output = np.empty((B, OUT), dtype=np.float32)
    uncertainty = np.empty((B, OUT), dtype=np.float32)
    for i in range(R):
        for j in range(C):
            c = i * C + j
            rsl = slice(i * BS, (i + 1) * BS)
            csl = slice(j * OS, (j + 1) * OS)
            output[rsl, csl] = res.results[c]["o_sh"].T.astype(np.float32)
            uncertainty[rsl, csl] = res.results[c]["u_sh"].T.astype(np.float32)
    return output, uncertainty


# revision 6
# speedup vs baseline: 1.6815x; 1.0119x over previous
"""Bayesian linear layer (reparameterized sample + predictive uncertainty)
as an 8-core SPMD Trainium2 Bass kernel.

Reference computation (all fp32):
    W     = weight_mu + exp(weight_log_sigma) * eps_w          # [OUT, IN]
    b     = bias_mu + exp(bias_log_sigma) * eps_b              # [OUT]
    out   = x @ W.T + b                                        # [B, OUT]
    unc   = sqrt((x*x) @ (exp(weight_log_sigma)**2).T + exp(bias_log_sigma)**2)

Strategy (v3):
  * Sharding: 4 batch-quarters x 2 out-feature-halves = 8 cores.
  * The host only re-lays-out inputs: transpose to contraction-major,
    downcast to bf16 (eps to fp8e4m3 - it enters W scaled by sigma~0.05,
    so fp8 noise is ~0.3% of W), tile into k-major blocks, and slice
    the shards. All arithmetic runs on device.
  * Matmul in bf16 (fp32 PSUM): out^T tiles = W-block.T @ x^T, weights
    stationary, x streaming, no device transposes. Weight blocks are
    (k,o)-contiguous so each lhsT is a contiguous 128x128 block; the
    two 512-col rhs blocks per (k,o) share one stationary load.
  * Inputs stream as ~1MB DMAs (4 chunks per tensor) on both HWDGE
    rings; on-device weight sampling (DVE) and x^2 (Act) follow the
    chunks at 2-k-tile granularity.
  * Fast path (constant weight_log_sigma): uncertainty collapses to
    sqrt(sigma^2 * rowsum(x^2) + bsig^2). rowsum(x^2) is computed on
    the PE with an ALL-ONES 128x128 stationary - every output
    partition receives the same column sum, i.e. the reduction arrives
    pre-broadcast in PSUM, and the uncertainty is a single Act pass
    (scale=sigma^2, per-partition bias=bsig^2, Sqrt) straight out of
    PSUM per o-tile.
  * Main matmuls run in 3 phases of <=6 concurrent PSUM groups, each
    group accumulating all 16 k in one bank (single DVE flush adds the
    bias and casts to bf16); within a phase the k loop is ascending so
    the PE consumes k-tiles in DMA-arrival order. The rs matmuls ride
    inside phase 1. Warm-up matmuls (read afterwards so DCE keeps
    them) cover the first-DMA latency and the PE HAM clock ramp.
  * Outputs are written as o^T / u^T bf16 shards; the host transposes
    and upcasts while assembling the full [B, OUT] fp32 arrays.
"""

import numpy as np
import ml_dtypes

B, IN, OUT = 4096, 2048, 2048
R, C = 4, 2              # batch split x out-feature split
N_CORES = R * C
BS = B // R              # 1024 batch rows per core
OS = OUT // C            # 1024 out features per core
KT = IN // 128           # 16 contraction k-tiles
OT = OS // 128           # 8 out-feature partition tiles per core
BB = BS // 512           # 2 psum column blocks
NWARM = 6
GCH = 4                  # k-tiles per input DMA chunk
VCH = 2                  # k-tiles per DVE/Act processing chunk

BF16 = ml_dtypes.bfloat16
FP8 = ml_dtypes.float8_e4m3

TRACE = False            # test harness sets True to capture an NTFF profile
LAST_RESULT = None       # BassKernelResults of the most recent run

_compiled = {}           # cache: key -> compiled Bass program


def _build(sigma_const):
    """Build + compile the per-core program. sigma_const=None -> general
    path (log_sigma streamed, second matmul for variance); float -> fast
    path with sigma baked in."""
    import concourse.mybir as mybir
    import concourse.tile as tile
    from concourse import bacc

    F32 = mybir.dt.float32
    BF = mybir.dt.bfloat16
    F8 = mybir.dt.float8e4
    AF = mybir.ActivationFunctionType
    ALU = mybir.AluOpType
    fast = sigma_const is not None

    nc = bacc.Bacc("TRN2", target_bir_lowering=False, debug=False,
                   num_devices=N_CORES)

    # weight-ish tensors are k-tile-major on the free axis; weight blocks
    # additionally o-tile-major: free col = (k*OT + o)*128 + c
    x_d = nc.dram_tensor("x_sh", [128, KT * BS], BF, kind="ExternalInput").ap()
    mu_d = nc.dram_tensor("mu_sh", [128, KT * OS], BF,
                          kind="ExternalInput").ap()
    eps_d = nc.dram_tensor("eps_sh", [128, KT * OS], F8 if fast else BF,
                           kind="ExternalInput").ap()
    if not fast:
        ls_d = nc.dram_tensor("ls_sh", [128, KT * OS], BF,
                              kind="ExternalInput").ap()
    bmu_d = nc.dram_tensor("bmu_sh", [128, OT], F32, kind="ExternalInput").ap()
    bls_d = nc.dram_tensor("bls_sh", [128, OT], F32, kind="ExternalInput").ap()
    beps_d = nc.dram_tensor("beps_sh", [128, OT], F32,
                            kind="ExternalInput").ap()
    o_d = nc.dram_tensor("o_sh", [OS, BS], BF, kind="ExternalOutput").ap()
    u_d = nc.dram_tensor("u_sh", [OS, BS], BF, kind="ExternalOutput").ap()

    with tile.TileContext(nc) as tc:
        with (
            tc.tile_pool(name="big", bufs=1) as big,
            tc.tile_pool(name="stage", bufs=2) as stage,
            tc.tile_pool(name="outs", bufs=3) as outs,
            tc.tile_pool(name="pmain", bufs=6, space="PSUM") as pmain,
            tc.tile_pool(name="paux", bufs=2, space="PSUM") as paux,
        ):
            # ---- resident SBUF tensors ----------------------------------
            xT = big.tile([128, KT * BS], BF, tag="xT")
            x2T = big.tile([128, KT * BS], BF, tag="x2T")
            wT = big.tile([128, KT * OS], BF, tag="wT")
            if not fast:
                s2T = big.tile([128, KT * OS], BF, tag="s2T")

            ones128 = big.tile([128, 128], BF, tag="ones128")
            nc.vector.memset(ones128[:], 1.0)
            rjunk = big.tile([128, 512], BF, tag="rjunk")
            nc.vector.memset(rjunk[:], 0.0)

            # warm-up matmuls: cover first-DMA latency + HAM clock ramp.
            # They write the rs PSUM tiles, which the real rs matmuls
            # clear via start=True and the u-path reads - so DCE keeps
            # them (a junk-only sink gets eliminated).
            prs = [paux.tile([128, 512], F32, tag="rs", bufs=2,
                             name="prs") for _ in range(BB)]
            for i in range(NWARM):
                nc.tensor.matmul(prs[i % BB][:], ones128[:], rjunk[:],
                                 start=True, stop=True)

            # ---- bias vectors as [128, OT] column grids -----------------
            bmu_sb = big.tile([128, OT], F32, tag="bmu")
            bls_sb = big.tile([128, OT], F32, tag="bls")
            beps_sb = big.tile([128, OT], F32, tag="beps")
            nc.sync.dma_start(bmu_sb[:], bmu_d[:])
            nc.sync.dma_start(bls_sb[:], bls_d[:])
            nc.sync.dma_start(beps_sb[:], beps_d[:])
            bsig = big.tile([128, OT], F32, tag="bsig")
            nc.scalar.activation(bsig[:], bls_sb[:], AF.Exp)
            bse = big.tile([128, OT], F32, tag="bse")
            nc.vector.tensor_tensor(bse[:], bsig[:], beps_sb[:], ALU.mult)
            bias_all = big.tile([128, OT], F32, tag="bias_all")
            nc.vector.tensor_tensor(bias_all[:], bmu_sb[:], bse[:], ALU.add)
            bs2_all = big.tile([128, OT], F32, tag="bs2_all")
            nc.vector.tensor_tensor(bs2_all[:], bsig[:], bsig[:], ALU.mult)

            # ---- chunked input DMAs + streaming prep --------------------
            # sync ring: mu (+ls); scalar ring: x, eps.
            CHUNKS = [1, 1, 2, 4, 4, 4]       # ramped k-tile chunks
            k0g = 0
            for cn in CHUNKS:
                xsl = slice(k0g * BS, (k0g + cn) * BS)
                wsl = slice(k0g * OS, (k0g + cn) * OS)
                nc.scalar.dma_start(xT[:, xsl], x_d[:, xsl])
                mu_t = stage.tile([128, 4 * OS], BF, tag="mu", bufs=2)
                nc.sync.dma_start(mu_t[:, :cn * OS], mu_d[:, wsl])
                eps_t = stage.tile([128, 4 * OS], F8 if fast else BF,
                                   tag="eps", bufs=2)
                nc.scalar.dma_start(eps_t[:, :cn * OS], eps_d[:, wsl])
                if not fast:
                    ls_t = stage.tile([128, 4 * OS], BF, tag="ls", bufs=2)
                    nc.sync.dma_start(ls_t[:, :cn * OS], ls_d[:, wsl])

                # per-VCH-chunk sampling / squares
                for v0 in range(0, cn, VCH):
                    vn = min(VCH, cn - v0)
                    lsl = slice(v0 * OS, (v0 + vn) * OS)           # in stage
                    gsl = slice((k0g + v0) * OS, (k0g + v0 + vn) * OS)
                    xvsl = slice((k0g + v0) * BS, (k0g + v0 + vn) * BS)
                    se_t = stage.tile([128, VCH * OS], BF, tag="se", bufs=2)
                    if fast:
                        nc.vector.tensor_scalar_mul(se_t[:, :vn * OS],
                                                    eps_t[:, lsl],
                                                    float(sigma_const))
                    else:
                        sig_t = stage.tile([128, VCH * OS], BF, tag="sig",
                                           bufs=2)
                        nc.scalar.activation(sig_t[:, :vn * OS], ls_t[:, lsl],
                                             AF.Exp)
                        nc.vector.tensor_tensor(se_t[:, :vn * OS],
                                                sig_t[:, :vn * OS],
                                                eps_t[:, lsl], ALU.mult)
                        nc.vector.tensor_tensor(s2T[:, gsl],
                                                sig_t[:, :vn * OS],
                                                sig_t[:, :vn * OS], ALU.mult)
                    nc.vector.tensor_tensor(wT[:, gsl], mu_t[:, lsl],
                                            se_t[:, :vn * OS], ALU.add)
                    nc.scalar.activation(x2T[:, xvsl], xT[:, xvsl], AF.Square)
                k0g += cn

            if fast:
                # rowsum(x^2) k-reduction tree on DVE (pairwise, no alias)
                lvl = [x2T[:, k * BS:(k + 1) * BS] for k in range(KT)]
                li = 0
                while len(lvl) > 1:
                    nxt = []
                    for p in range(0, len(lvl) - 1, 2):
                        if len(lvl) == 2:
                            dst = big.tile([128, BS], BF, tag="xsum")
                        else:
                            dst = big.tile([128, BS], BF, tag=f"ts{li}_{p}",
                                           name=f"ts{li}_{p}")
                        nc.vector.tensor_tensor(dst[:], lvl[p], lvl[p + 1],
                                                ALU.add)
                        nxt.append(dst[:])
                    if len(lvl) % 2:
                        nxt.append(lvl[-1])
                    lvl = nxt
                    li += 1
                xsum = lvl[0]

            def w_blk(k, o):
                c = (k * OT + o) * 128
                return wT[:, c:c + 128]

            def s2_blk(k, o):
                c = (k * OT + o) * 128
                return s2T[:, c:c + 128]

            def x_sl(k, bb):
                c = k * BS + bb * 512
                return xT[:, c:c + 512]

            def x2_sl(k, bb):
                c = k * BS + bb * 512
                return x2T[:, c:c + 512]

            if fast:
                # ---------------- fast path ------------------------------
                def phase(os_list, with_rs):
                    pos = {}
                    ots = {}
                    for o in os_list:
                        ots[o] = outs.tile([128, BS], BF, tag="o", bufs=3,
                                           name="ot")
                        for bb in range(BB):
                            pos[(o, bb)] = pmain.tile([128, 512], F32,
                                                      tag="po", name="po")
                    for k in range(KT):
                        for o in os_list:
                            for bb in range(BB):
                                nc.tensor.matmul(pos[(o, bb)][:], w_blk(k, o),
                                                 x_sl(k, bb),
                                                 start=(k == 0),
                                                 stop=(k == KT - 1))
                    if with_rs:
                        for bb in range(BB):
                            nc.tensor.matmul(prs[bb][:], ones128[:],
                                             xsum[:, bb * 512:(bb + 1) * 512],
                                             start=True, stop=True)
                    for o in os_list:
                        for bb in range(BB):
                            bsl = slice(bb * 512, (bb + 1) * 512)
                            nc.vector.tensor_scalar_add(ots[o][:, bsl],
                                                        pos[(o, bb)][:],
                                                        bias_all[:, o:o + 1])
                            nc.sync.dma_start(
                                o_d[o * 128:(o + 1) * 128, bsl],
                                ots[o][:, bsl])

                phase([0, 1], with_rs=True)

                # u^T = sqrt(sigma^2 * rs + bsig^2[o]) straight out of PSUM
                s2 = float(sigma_const) * float(sigma_const)
                for o in range(OT):
                    ut = outs.tile([128, BS], BF, tag="u", bufs=3, name="ut")
                    for bb in range(BB):
                        bsl = slice(bb * 512, (bb + 1) * 512)
                        nc.scalar.activation(ut[:, bsl], prs[bb][:], AF.Sqrt,
                                             scale=s2,
                                             bias=bs2_all[:, o:o + 1])
                    nc.scalar.dma_start(u_d[o * 128:(o + 1) * 128, :], ut[:])

                phase([2, 3, 4], with_rs=False)
                phase([5, 6, 7], with_rs=False)
            else:
                # ---------------- general path ---------------------------
                for o in range(OT):
                    ot = outs.tile([128, BS], BF, tag="o", bufs=3, name="ot")
                    ut = outs.tile([128, BS], BF, tag="u", bufs=3, name="ut")
                    for bb in range(BB):
                        bsl = slice(bb * 512, (bb + 1) * 512)
                        po = pmain.tile([128, 512], F32, tag="po", name="po")
                        for k in range(KT):
                            nc.tensor.matmul(po[:], w_blk(k, o), x_sl(k, bb),
                                             start=(k == 0),
                                             stop=(k == KT - 1))
                        nc.vector.tensor_scalar_add(ot[:, bsl], po[:],
                                                    bias_all[:, o:o + 1])
                        pu = pmain.tile([128, 512], F32, tag="po", name="pu")
                        for k in range(KT):
                            nc.tensor.matmul(pu[:], s2_blk(k, o),
                                             x2_sl(k, bb),
                                             start=(k == 0),
                                             stop=(k == KT - 1))
                        nc.scalar.activation(ut[:, bsl], pu[:], AF.Sqrt,
                                             bias=bs2_all[:, o:o + 1])
                    nc.sync.dma_start(o_d[o * 128:(o + 1) * 128, :], ot[:])
                    nc.scalar.dma_start(u_d[o * 128:(o + 1) * 128, :], ut[:])

    nc.compile()
    return nc


def _ktile_major(aT, width):
    """[IN, W] (contraction-major) -> [128, KT*W] with k-tile t at free
    cols [t*W, (t+1)*W)."""
    return np.ascontiguousarray(
        aT.reshape(KT, 128, width).transpose(1, 0, 2).reshape(128, KT * width))


def _weight_blocks(aT):
    """[IN, OS] -> [128, KT*OS] with contiguous 128-wide (k,o) blocks:
    free col = (k*OT + o)*128 + c."""
    return np.ascontiguousarray(
        aT.reshape(KT, 128, OT, 128).transpose(1, 0, 2, 3).reshape(
            128, KT * OS))


def _bias_grid(v):
    """[OS] fp32 slice -> [128, OT] grid with o-tile t in column t."""
    return np.ascontiguousarray(
        np.asarray(v, dtype=np.float32).reshape(OT, 128).T)


def kernel(x, weight_mu, weight_log_sigma, bias_mu, bias_log_sigma,
           eps_w, eps_b):
    global LAST_RESULT
    from concourse.bass_utils import run_bass_kernel_spmd

    x = np.asarray(x, dtype=np.float32)
    weight_mu = np.asarray(weight_mu, dtype=np.float32)
    weight_log_sigma = np.asarray(weight_log_sigma, dtype=np.float32)
    bias_mu = np.asarray(bias_mu, dtype=np.float32)
    bias_log_sigma = np.asarray(bias_log_sigma, dtype=np.float32)
    eps_w = np.asarray(eps_w, dtype=np.float32)
    eps_b = np.asarray(eps_b, dtype=np.float32)

    ls0 = weight_log_sigma.flat[0]
    fast = bool(np.all(weight_log_sigma == ls0))
    sigma_const = float(np.exp(np.float32(ls0))) if fast else None

    key = ("fast", sigma_const) if fast else ("general",)
    if key not in _compiled:
        _compiled[key] = _build(sigma_const)
    nc = _compiled[key]

    # host-side layout: transpose to contraction-major, downcast, tile
    xT = x.astype(BF16).T                                    # [IN, B] view
    muT = weight_mu.astype(BF16).T                           # [IN, OUT]
    epsT = eps_w.astype(FP8 if fast else BF16).T
    if not fast:
        lsT = weight_log_sigma.astype(BF16).T

    in_maps = []
    for i in range(R):
        for j in range(C):
            osl = slice(j * OS, (j + 1) * OS)
            m = {
                "x_sh": _ktile_major(
                    np.ascontiguousarray(xT[:, i * BS:(i + 1) * BS]), BS),
                "mu_sh": _weight_blocks(np.ascontiguousarray(muT[:, osl])),
                "eps_sh": _weight_blocks(np.ascontiguousarray(epsT[:, osl])),
                "bmu_sh": _bias_grid(bias_mu[osl]),
                "bls_sh": _bias_grid(bias_log_sigma[osl]),
                "beps_sh": _bias_grid(eps_b[osl]),
            }
            if not fast:
                m["ls_sh"] = _weight_blocks(np.ascontiguousarray(lsT[:, osl]))
            in_maps.append(m)

    res = run_bass_kernel_spmd(nc, in_maps, core_ids=list(range(N_CORES)),
                               trace=TRACE)
    LAST_RESULT = res

    output = np.empty((B, OUT), dtype=np.float32)
    uncertainty = np.empty((B, OUT), dtype=np.float32)
    for i in range(R):
        for j in range(C):
            c = i * C + j
            rsl = slice(i * BS, (i + 1) * BS)
            csl = slice(j * OS, (j + 1) * OS)
            output[rsl, csl] = res.results[c]["o_sh"].T.astype(np.float32)
            uncertainty[rsl, csl] = res.results[c]["u_sh"].T.astype(np.float32)
    return output, uncertainty
